# revision 15
# baseline (speedup 1.0000x reference)
"""Trainium2 Bass kernel for nn_DGCRM (GRU-style recurrent graph model).

Math per step t (per batch b):
  inp  = [x_t, state]                                  # [N, 66]
  zr   = sigmoid(inp @ Wg_al + bg_al + s0g * agg_g)    # agg_g = sum_n relu(inp @ Wg + bg)
  z, r = zr[:, :64], zr[:, 64:]
  cand = [x_t, z * state]
  hc   = tanh(cand @ Wu_al + bu_al + s0u * agg_u)      # agg_u = sum_n relu(cand @ Wu + bu)
  state = r * state + (1 - r) * hc

The rank-1 "diffusion" term s[n] * agg[c] collapses to a per-channel bias
because setup_inputs() uses uniform node/add weights (s[n] == s0) and
aff_w == 1, aff_b == 0 (verified at runtime; general numpy fallback
otherwise).

Device layout is channel-major and fully batch-packed: every [128, N]
SBUF tensor holds batch b0 on partitions 0:64 and b1 on 64:128, so all
elementwise work runs at full 128-lane width. Matmuls are M=64 per batch,
issued as concurrent tile_position pairs (b0 in array rows/cols 0:64, b1
in 64:128), with the x_t/bias contribution added via a separate K=3
accumulating matmul (x rows + a const-1 row carry the bias). The node dim
streams through PE in <=512-column slices. Sharding: data-parallel over
batch, 2 batches per core, no collectives. Output is written channel-major
[b, t, 64, N] and transposed to [b, t, N, 64] on the host during unshard.

PSUM banks (each [128, chunk=1024] fp32 = 2 banks):
  Hz/Hr: relu(inp@Wg+bg) halves -> node-sum only (agg)
  Z/R:   inp@Wg_al halves -> sigmoid -> z (scratch) / r (persistent)
  U1:    cand@Wu -> relu-sum; U2: cand@Wu_al -> tanh -> hc
"""

import os
import numpy as np

N = 8600
DIN = 2
H = 64
B = 16
T = 12
NCORES = 8
BPC = B // NCORES  # batches per core

# bank indices in the packed weight tensors
HZ, HR, ZB, RB, U1, U2 = range(6)


# ---------------------------------------------------------------------------
# device program
# ---------------------------------------------------------------------------

def _build_program(n, t_steps, chunk, s0g, s0u):
    import concourse.bacc as bacc
    from concourse import mybir
    import concourse.tile as tile_mod

    f32 = mybir.dt.float32
    AF = mybir.ActivationFunctionType
    ALU = mybir.AluOpType

    chunks = []
    c0 = 0
    while c0 < n:
        chunks.append((c0, min(chunk, n - c0)))
        c0 += chunk
    nch = len(chunks)

    def mm_slices(cw):
        out, s = [], 0
        while s < cw:
            out.append((s, min(512, cw - s)))
            s += 512
        return out

    nc = bacc.Bacc("TRN2", target_bir_lowering=False, debug=False)

    bf16_d = mybir.dt.bfloat16
    xt_d = nc.dram_tensor("xt", [BPC, t_steps, DIN, n], bf16_d, kind="ExternalInput")
    st0_d = nc.dram_tensor("st0", [BPC, H, n], bf16_d, kind="ExternalInput")
    wst_d = nc.dram_tensor("wst", [64, 6, 64], bf16_d, kind="ExternalInput")
    wx1_d = nc.dram_tensor("wx1", [3, 6, 64], bf16_d, kind="ExternalInput")
    out_d = nc.dram_tensor("out_cm", [BPC, t_steps, H, n], bf16_d, kind="ExternalOutput")

    bf16 = mybir.dt.bfloat16

    with tile_mod.TileContext(nc) as tc:
        with (
            tc.tile_pool(name="persist", bufs=1) as persist,
            tc.tile_pool(name="psum", bufs=4, space="PSUM") as psum,
            tc.tile_pool(name="zpool", bufs=3) as zpool,
            tc.tile_pool(name="hcpool", bufs=3) as hcpool,
            tc.tile_pool(name="dpool", bufs=3) as dpool,
            tc.tile_pool(name="smalls", bufs=2) as smalls,
        ):
            Spack = persist.tile([128, n], bf16, name="Spack")
            Rpack = persist.tile([128, n], bf16, name="Rpack")
            Cpack = persist.tile([128, n], bf16, name="Cpack")
            # x + const-1 rows, double-buffered by step parity at partition
            # offsets (0,32) / (64,96) so next-next-step x DMAs never touch
            # rows the current step reads.
            X = persist.tile([99, n], bf16, name="X")
            # weights: state parts duplicated at partition 0 and 64 (the two
            # concurrent-matmul positions); x+bias parts at all 4 x slots.
            wst = persist.tile([128, 6, 64], bf16, name="wst")
            wx1 = persist.tile([99, 6, 64], bf16, name="wx1")

            nc.sync.dma_start(wst[0:64], wst_d[:])
            nc.sync.dma_start(wst[64:128], wst_d[:])
            xslots = (0, 32, 64, 96)
            for r in xslots:
                nc.sync.dma_start(wx1[r:r + 3], wx1_d[:])
                nc.vector.memset(X[r:r + 3, :], 1.0)

            nc.sync.dma_start(Spack[0:64, :], st0_d[0])
            nc.sync.dma_start(Spack[64:128, :], st0_d[1])

            def dma_x(t):
                r0, r1 = (0, 32) if t % 2 == 0 else (64, 96)
                nc.sync.dma_start(X[r0:r0 + 2, :], xt_d[0, t])
                nc.sync.dma_start(X[r1:r1 + 2, :], xt_d[1, t])

            dma_x(0)
            if t_steps > 1:
                dma_x(1)

            def mm_bank(ps, k, cs, cw, rhs, t):
                """Fill ps[128, cw] with bank k: rows 0:64 = b0, 64:128 = b1;
                x/bias part (K=3) accumulates with the state part (K=64).
                bf16 moving operand: up to 1024 cols per matmul."""
                r0, r1 = (0, 32) if t % 2 == 0 else (64, 96)
                sls = mm_slices(cw)
                for ss, sw in sls:
                    sl = slice(cs + ss, cs + ss + sw)
                    nc.tensor.matmul(ps[0:64, ss:ss + sw], wx1[r0:r0 + 3, k, :],
                                     X[r0:r0 + 3, sl],
                                     start=True, stop=False,
                                     skip_group_check=True,
                                     tile_position=(r0, 0))
                for ss, sw in sls:
                    sl = slice(cs + ss, cs + ss + sw)
                    nc.tensor.matmul(ps[64:128, ss:ss + sw], wx1[r1:r1 + 3, k, :],
                                     X[r1:r1 + 3, sl],
                                     start=True, stop=False,
                                     skip_group_check=True,
                                     tile_position=(r1, 64))
                for ss, sw in sls:
                    sl = slice(cs + ss, cs + ss + sw)
                    nc.tensor.matmul(ps[0:64, ss:ss + sw], wst[0:64, k, :],
                                     rhs[0:64, sl],
                                     start=False, stop=True,
                                     skip_group_check=True,
                                     tile_position=(0, 0))
                for ss, sw in sls:
                    sl = slice(cs + ss, cs + ss + sw)
                    nc.tensor.matmul(ps[64:128, ss:ss + sw], wst[64:128, k, :],
                                     rhs[64:128, sl],
                                     start=False, stop=True,
                                     skip_group_check=True,
                                     tile_position=(64, 64))

            def act_bias(parts, scale, name):
                """bias[128,1] = scale * sum(parts) in ONE ScalarE op so the
                relu-partials -> bias -> sigmoid chain stays on-engine."""
                dummy = smalls.tile([128, nch], f32, tag="bdummy",
                                    name=f"bd_{name}")
                bias = smalls.tile([128, 1], f32, tag="bias",
                                   name=f"bias_{name}")
                nc.scalar.activation(dummy[:, 0:nch], parts[:, 0:nch],
                                     AF.Identity, scale=float(scale),
                                     accum_out=bias[:, 0:1])
                return bias

            for t in range(t_steps):
                # ---- gate_w: agg_g halves (relu in place on PSUM) ---------
                pHz = smalls.tile([128, nch], f32, tag="pHz", name=f"pHz_{t}")
                pHr = smalls.tile([128, nch], f32, tag="pHr", name=f"pHr_{t}")
                for ci, (cs, cw) in enumerate(chunks):
                    for k, parts in ((HZ, pHz), (HR, pHr)):
                        ps = psum.tile([128, chunk], f32, tag="ps",
                                       name=f"ps_g{k}_{t}_{ci}")
                        mm_bank(ps, k, cs, cw, Spack, t)
                        if k == HZ:  # balance: Hz relu+sum on DVE, Hr on ACT
                            nc.vector.tensor_scalar(
                                ps[:, :cw], ps[:, :cw], 0.0, None, ALU.max,
                                ALU.add, accum_out=parts[:, ci:ci + 1])
                        else:
                            nc.scalar.activation(ps[:, :cw], ps[:, :cw],
                                                 AF.Relu,
                                                 accum_out=parts[:, ci:ci + 1])
                biasZ = act_bias(pHz, s0g, f"Z_{t}")
                biasR = act_bias(pHr, s0g, f"R_{t}")

                # ---- gate_align -> sigmoid -> z/r; cand = z*state ---------
                for ci, (cs, cw) in enumerate(chunks):
                    psZ = psum.tile([128, chunk], f32, tag="ps",
                                    name=f"ps_z_{t}_{ci}")
                    mm_bank(psZ, ZB, cs, cw, Spack, t)
                    zc = zpool.tile([128, chunk], bf16, tag="zc",
                                    name=f"zc_{t}_{ci}")
                    nc.scalar.activation(zc[:, :cw], psZ[:, :cw], AF.Sigmoid,
                                         bias=biasZ[:, 0:1])
                    psR = psum.tile([128, chunk], f32, tag="ps",
                                    name=f"ps_r_{t}_{ci}")
                    mm_bank(psR, RB, cs, cw, Spack, t)
                    nc.scalar.activation(Rpack[:, cs:cs + cw], psR[:, :cw],
                                         AF.Sigmoid, bias=biasR[:, 0:1])
                    nc.vector.tensor_mul(Cpack[:, cs:cs + cw], zc[:, :cw],
                                         Spack[:, cs:cs + cw])

                # ---- upd_w: agg_u -----------------------------------------
                pU = smalls.tile([128, nch], f32, tag="pU", name=f"pU_{t}")
                for ci, (cs, cw) in enumerate(chunks):
                    ps = psum.tile([128, chunk], f32, tag="ps",
                                   name=f"ps_u1_{t}_{ci}")
                    mm_bank(ps, U1, cs, cw, Cpack, t)
                    nc.vector.tensor_scalar(ps[:, :cw], ps[:, :cw], 0.0, None,
                                            ALU.max, ALU.add,
                                            accum_out=pU[:, ci:ci + 1])
                biasU = act_bias(pU, s0u, f"U_{t}")

                # ---- upd_align -> tanh -> state update --------------------
                for ci, (cs, cw) in enumerate(chunks):
                    ps = psum.tile([128, chunk], f32, tag="ps",
                                   name=f"ps_u2_{t}_{ci}")
                    mm_bank(ps, U2, cs, cw, Cpack, t)
                    hc = hcpool.tile([128, chunk], bf16, tag="hc",
                                     name=f"hc_{t}_{ci}")
                    nc.scalar.activation(hc[:, :cw], ps[:, :cw], AF.Tanh,
                                         bias=biasU[:, 0:1])
                    # state = hc + r * (state - hc)
                    dd = dpool.tile([128, chunk], bf16, tag="dd",
                                    name=f"dd_{t}_{ci}")
                    nc.vector.tensor_sub(dd[:, :cw], Spack[:, cs:cs + cw],
                                         hc[:, :cw])
                    nc.vector.tensor_mul(dd[:, :cw], Rpack[:, cs:cs + cw],
                                         dd[:, :cw])
                    nc.vector.tensor_add(Spack[:, cs:cs + cw], hc[:, :cw],
                                         dd[:, :cw])

                nc.sync.dma_start(out_d[0, t], Spack[0:64, :])
                nc.sync.dma_start(out_d[1, t], Spack[64:128, :])
                # prefetch x two steps ahead (same parity slots, whose
                # readers in this step are done by now)
                if t + 2 < t_steps:
                    dma_x(t + 2)

    nc.compile()
    return nc


_PROG_CACHE = {}


def _get_program(n, t_steps, chunk, s0g, s0u):
    key = (n, t_steps, chunk, float(s0g), float(s0u))
    if key not in _PROG_CACHE:
        _PROG_CACHE[key] = _build_program(n, t_steps, chunk, s0g, s0u)
    return _PROG_CACHE[key]


# ---------------------------------------------------------------------------
# host-side packing / fallback
# ---------------------------------------------------------------------------

def _pack_weights(kw):
    """wst [64, 6, 64] (state rows) / wx1 [3, 6, 64] (x rows + bias row).
    Banks: Hz, Hr = gate_w halves; Z, R = gate_align halves; U1 = upd_w;
    U2 = upd_align."""
    wst = np.empty((64, 6, 64), np.float32)
    wx1 = np.empty((3, 6, 64), np.float32)
    specs = [
        (HZ, kw['gate_w'][:, 0:64], kw['gate_b'][0:64]),
        (HR, kw['gate_w'][:, 64:128], kw['gate_b'][64:128]),
        (ZB, kw['gate_align_w'][:, 0:64], kw['gate_align_b'][0:64]),
        (RB, kw['gate_align_w'][:, 64:128], kw['gate_align_b'][64:128]),
        (U1, kw['upd_w'], kw['upd_b']),
        (U2, kw['upd_align_w'], kw['upd_align_b']),
    ]
    for k, w, bias in specs:
        wst[:, k, :] = w[DIN:]
        wx1[0:DIN, k, :] = w[:DIN]
        wx1[DIN, k, :] = bias
    return wst, wx1


def _numpy_reference(x, init_state, kw):
    """Faithful general fallback (and testing oracle)."""
    def gfs(inp, aw, ab, w, b, nw, adw, afw, afb):
        res = inp @ aw + ab
        h = np.maximum(inp @ w + b, 0.0)
        s = adw[:, 0] * nw[0, :]
        agg = h.sum(axis=1)
        diff = s[None, :, None] * agg[:, None, :]
        return res + afw * diff + afb

    state = init_state[0].astype(np.float32)
    states = []
    for t in range(x.shape[1]):
        xt = x[:, t]
        inp = np.concatenate([xt, state], axis=-1)
        zr = 1.0 / (1.0 + np.exp(-gfs(inp, kw['gate_align_w'], kw['gate_align_b'],
                                      kw['gate_w'], kw['gate_b'], kw['gate_node_w'],
                                      kw['gate_add_w'], kw['gate_aff_w'],
                                      kw['gate_aff_b'])))
        z, r = zr[..., :H], zr[..., H:]
        cand = np.concatenate([xt, z * state], axis=-1)
        hc = np.tanh(gfs(cand, kw['upd_align_w'], kw['upd_align_b'],
                         kw['upd_w'], kw['upd_b'], kw['upd_node_w'],
                         kw['upd_add_w'], kw['upd_aff_w'], kw['upd_aff_b']))
        state = r * state + (1.0 - r) * hc
        states.append(state.copy())
    out = np.stack(states, axis=1).astype(np.float32)
    return out, np.ascontiguousarray(out[:, -1])


def kernel(**inputs):
    x = np.ascontiguousarray(np.asarray(inputs['x'], np.float32))
    init_state = np.asarray(inputs['init_state'], np.float32)
    kw = {k: np.asarray(v, np.float32) for k, v in inputs.items()
          if k not in ('x', 'init_state', 'node_emb0', 'node_emb1')}

    b_, t_, n_, d_ = x.shape

    s_g = kw['gate_add_w'][:, 0] * kw['gate_node_w'][0, :]
    s_u = kw['upd_add_w'][:, 0] * kw['upd_node_w'][0, :]

    def _uniform(v):
        return np.all(v == v.flat[0])

    structured = (
        b_ == B and t_ == T and n_ == N and d_ == DIN
        and _uniform(s_g) and _uniform(s_u)
        and np.all(kw['gate_aff_w'] == 1.0) and np.all(kw['gate_aff_b'] == 0.0)
        and np.all(kw['upd_aff_w'] == 1.0) and np.all(kw['upd_aff_b'] == 0.0)
    )
    if not structured:
        return _numpy_reference(x, init_state, kw)

    s0g = float(s_g.flat[0])
    s0u = float(s_u.flat[0])

    import ml_dtypes
    bf = ml_dtypes.bfloat16
    wst, wx1 = _pack_weights(kw)
    wst = wst.astype(bf)
    wx1 = wx1.astype(bf)

    xt_all = np.ascontiguousarray(x.transpose(0, 1, 3, 2)).astype(bf)
    st0_all = np.ascontiguousarray(init_state[0].transpose(0, 2, 1)).astype(bf)

    chunk = 1024
    nc = _get_program(N, T, chunk, s0g, s0u)

    from concourse.bass_utils import run_bass_kernel_spmd
    in_maps = []
    for c in range(NCORES):
        sl = slice(BPC * c, BPC * (c + 1))
        in_maps.append(dict(xt=xt_all[sl], st0=st0_all[sl], wst=wst, wx1=wx1))

    trace = os.environ.get("DGCRM_TRACE", "0") == "1"
    res = run_bass_kernel_spmd(nc, in_maps, core_ids=list(range(NCORES)),
                               trace=trace)
    if trace and res.exec_time_ns is not None:
        kernel._last_exec_time_ns = res.exec_time_ns
        kernel._last_trace = res.instructions_and_trace
    out_cm = np.concatenate([r["out_cm"] for r in res.results], axis=0)  # [B,T,64,N]
    states = np.ascontiguousarray(out_cm.transpose(0, 1, 3, 2).astype(np.float32))
    last = np.ascontiguousarray(states[:, -1])
    return states, last


kernel._last_exec_time_ns = None
kernel._last_trace = None


# revision 16
# speedup vs baseline: 1.0395x; 1.0395x over previous
"""Trainium2 Bass kernel for nn_DGCRM (GRU-style recurrent graph model).

Math per step t (per batch b):
  inp  = [x_t, state]                                  # [N, 66]
  zr   = sigmoid(inp @ Wg_al + bg_al + s0g * agg_g)    # agg_g = sum_n relu(inp @ Wg + bg)
  z, r = zr[:, :64], zr[:, 64:]
  cand = [x_t, z * state]
  hc   = tanh(cand @ Wu_al + bu_al + s0u * agg_u)      # agg_u = sum_n relu(cand @ Wu + bu)
  state = r * state + (1 - r) * hc

The rank-1 "diffusion" term s[n] * agg[c] collapses to a per-channel bias
because setup_inputs() uses uniform node/add weights (s[n] == s0) and
aff_w == 1, aff_b == 0 (verified at runtime; general numpy fallback
otherwise).

Device layout is channel-major and fully batch-packed: every [128, N]
SBUF tensor holds batch b0 on partitions 0:64 and b1 on 64:128, so all
elementwise work runs at full 128-lane width. Matmuls are M=64 per batch,
issued as concurrent tile_position pairs (b0 in array rows/cols 0:64, b1
in 64:128), with the x_t/bias contribution added via a separate K=3
accumulating matmul (x rows + a const-1 row carry the bias). The node dim
streams through PE in <=512-column slices. Sharding: data-parallel over
batch, 2 batches per core, no collectives. Output is written channel-major
[b, t, 64, N] and transposed to [b, t, N, 64] on the host during unshard.

PSUM banks (each [128, chunk=1024] fp32 = 2 banks):
  Hz/Hr: relu(inp@Wg+bg) halves -> node-sum only (agg)
  Z/R:   inp@Wg_al halves -> sigmoid -> z (scratch) / r (persistent)
  U1:    cand@Wu -> relu-sum; U2: cand@Wu_al -> tanh -> hc
"""

import os
import numpy as np

N = 8600
DIN = 2
H = 64
B = 16
T = 12
NCORES = 8
BPC = B // NCORES  # batches per core

# bank indices in the packed weight tensors
HZ, HR, ZB, RB, U1, U2 = range(6)


# ---------------------------------------------------------------------------
# device program
# ---------------------------------------------------------------------------

def _build_program(n, t_steps, chunk, s0g, s0u):
    import concourse.bacc as bacc
    from concourse import mybir
    import concourse.tile as tile_mod

    f32 = mybir.dt.float32
    AF = mybir.ActivationFunctionType
    ALU = mybir.AluOpType

    chunks = []
    c0 = 0
    while c0 < n:
        chunks.append((c0, min(chunk, n - c0)))
        c0 += chunk
    nch = len(chunks)

    def mm_slices(cw):
        out, s = [], 0
        while s < cw:
            out.append((s, min(512, cw - s)))
            s += 512
        return out

    nc = bacc.Bacc("TRN2", target_bir_lowering=False, debug=False)

    bf16_d = mybir.dt.bfloat16
    xt_d = nc.dram_tensor("xt", [BPC, t_steps, DIN, n], bf16_d, kind="ExternalInput")
    st0_d = nc.dram_tensor("st0", [BPC, H, n], bf16_d, kind="ExternalInput")
    wst_d = nc.dram_tensor("wst", [64, 6, 64], bf16_d, kind="ExternalInput")
    wx1_d = nc.dram_tensor("wx1", [3, 6, 64], bf16_d, kind="ExternalInput")
    out_d = nc.dram_tensor("out_cm", [BPC, t_steps, H, n], bf16_d, kind="ExternalOutput")

    bf16 = mybir.dt.bfloat16

    with tile_mod.TileContext(nc) as tc:
        with (
            tc.tile_pool(name="persist", bufs=1) as persist,
            tc.tile_pool(name="psum", bufs=4, space="PSUM") as psum,
            tc.tile_pool(name="zpool", bufs=4) as zpool,
            tc.tile_pool(name="hcpool", bufs=4) as hcpool,
            tc.tile_pool(name="dpool", bufs=4) as dpool,
            tc.tile_pool(name="smalls", bufs=3) as smalls,
        ):
            Spack = persist.tile([128, n], bf16, name="Spack")
            Rpack = persist.tile([128, n], bf16, name="Rpack")
            Cpack = persist.tile([128, n], bf16, name="Cpack")
            # x + const-1 rows, double-buffered by step parity at partition
            # offsets (0,32) / (64,96) so next-next-step x DMAs never touch
            # rows the current step reads.
            X = persist.tile([99, n], bf16, name="X")
            # weights: state parts duplicated at partition 0 and 64 (the two
            # concurrent-matmul positions); x+bias parts at all 4 x slots.
            wst = persist.tile([128, 6, 64], bf16, name="wst")
            wx1 = persist.tile([99, 6, 64], bf16, name="wx1")

            nc.sync.dma_start(wst[0:64], wst_d[:])
            nc.sync.dma_start(wst[64:128], wst_d[:])
            xslots = (0, 32, 64, 96)
            for r in xslots:
                nc.sync.dma_start(wx1[r:r + 3], wx1_d[:])
                nc.vector.memset(X[r:r + 3, :], 1.0)

            nc.sync.dma_start(Spack[0:64, :], st0_d[0])
            nc.sync.dma_start(Spack[64:128, :], st0_d[1])

            def dma_x(t):
                r0, r1 = (0, 32) if t % 2 == 0 else (64, 96)
                nc.sync.dma_start(X[r0:r0 + 2, :], xt_d[0, t])
                nc.sync.dma_start(X[r1:r1 + 2, :], xt_d[1, t])

            dma_x(0)
            if t_steps > 1:
                dma_x(1)

            def mm_bank(ps, k, cs, cw, rhs, t):
                """Fill ps[128, cw] with bank k: rows 0:64 = b0, 64:128 = b1;
                x/bias part (K=3) accumulates with the state part (K=64).
                bf16 moving operand: up to 1024 cols per matmul."""
                r0, r1 = (0, 32) if t % 2 == 0 else (64, 96)
                sls = mm_slices(cw)
                for ss, sw in sls:
                    sl = slice(cs + ss, cs + ss + sw)
                    nc.tensor.matmul(ps[0:64, ss:ss + sw], wx1[r0:r0 + 3, k, :],
                                     X[r0:r0 + 3, sl],
                                     start=True, stop=False,
                                     skip_group_check=True,
                                     tile_position=(r0, 0))
                for ss, sw in sls:
                    sl = slice(cs + ss, cs + ss + sw)
                    nc.tensor.matmul(ps[64:128, ss:ss + sw], wx1[r1:r1 + 3, k, :],
                                     X[r1:r1 + 3, sl],
                                     start=True, stop=False,
                                     skip_group_check=True,
                                     tile_position=(r1, 64))
                for ss, sw in sls:
                    sl = slice(cs + ss, cs + ss + sw)
                    nc.tensor.matmul(ps[0:64, ss:ss + sw], wst[0:64, k, :],
                                     rhs[0:64, sl],
                                     start=False, stop=True,
                                     skip_group_check=True,
                                     tile_position=(0, 0))
                for ss, sw in sls:
                    sl = slice(cs + ss, cs + ss + sw)
                    nc.tensor.matmul(ps[64:128, ss:ss + sw], wst[64:128, k, :],
                                     rhs[64:128, sl],
                                     start=False, stop=True,
                                     skip_group_check=True,
                                     tile_position=(64, 64))

            def act_bias(parts, scale, name):
                """bias[128,1] = scale * sum(parts) in ONE ScalarE op so the
                relu-partials -> bias -> sigmoid chain stays on-engine."""
                dummy = smalls.tile([128, nch], f32, tag="bdummy",
                                    name=f"bd_{name}")
                bias = smalls.tile([128, 1], f32, tag="bias",
                                   name=f"bias_{name}")
                nc.scalar.activation(dummy[:, 0:nch], parts[:, 0:nch],
                                     AF.Identity, scale=float(scale),
                                     accum_out=bias[:, 0:1])
                return bias

            def gate_w_chunk(t, ci, cs, cw, pHz, pHr):
                for k, parts in ((HZ, pHz), (HR, pHr)):
                    ps = psum.tile([128, chunk], f32, tag="ps",
                                   name=f"ps_g{k}_{t}_{ci}")
                    mm_bank(ps, k, cs, cw, Spack, t)
                    if k == HZ:  # balance: Hz relu+sum on DVE, Hr on ACT
                        nc.vector.tensor_scalar(
                            ps[:, :cw], ps[:, :cw], 0.0, None, ALU.max,
                            ALU.add, accum_out=parts[:, ci:ci + 1])
                    else:
                        nc.scalar.activation(ps[:, :cw], ps[:, :cw], AF.Relu,
                                             accum_out=parts[:, ci:ci + 1])

            def new_partials(t):
                pHz = smalls.tile([128, nch], f32, tag="pHz", name=f"pHz_{t}")
                pHr = smalls.tile([128, nch], f32, tag="pHr", name=f"pHr_{t}")
                return pHz, pHr

            # gate_w for step 0 stands alone; afterwards gate_w(t+1) is
            # fused into step t's update loop chunk-by-chunk.
            pHz, pHr = new_partials(0)
            for ci, (cs, cw) in enumerate(chunks):
                gate_w_chunk(0, ci, cs, cw, pHz, pHr)

            for t in range(t_steps):
                biasZ = act_bias(pHz, s0g, f"Z_{t}")
                biasR = act_bias(pHr, s0g, f"R_{t}")

                # ---- gate_align -> sigmoid -> z/r; cand = z*state ---------
                for ci, (cs, cw) in enumerate(chunks):
                    psZ = psum.tile([128, chunk], f32, tag="ps",
                                    name=f"ps_z_{t}_{ci}")
                    mm_bank(psZ, ZB, cs, cw, Spack, t)
                    zc = zpool.tile([128, chunk], bf16, tag="zc",
                                    name=f"zc_{t}_{ci}")
                    nc.scalar.activation(zc[:, :cw], psZ[:, :cw], AF.Sigmoid,
                                         bias=biasZ[:, 0:1])
                    psR = psum.tile([128, chunk], f32, tag="ps",
                                    name=f"ps_r_{t}_{ci}")
                    mm_bank(psR, RB, cs, cw, Spack, t)
                    nc.scalar.activation(Rpack[:, cs:cs + cw], psR[:, :cw],
                                         AF.Sigmoid, bias=biasR[:, 0:1])
                    nc.vector.tensor_mul(Cpack[:, cs:cs + cw], zc[:, :cw],
                                         Spack[:, cs:cs + cw])

                # ---- upd_w: agg_u -----------------------------------------
                pU = smalls.tile([128, nch], f32, tag="pU", name=f"pU_{t}")
                for ci, (cs, cw) in enumerate(chunks):
                    ps = psum.tile([128, chunk], f32, tag="ps",
                                   name=f"ps_u1_{t}_{ci}")
                    mm_bank(ps, U1, cs, cw, Cpack, t)
                    nc.vector.tensor_scalar(ps[:, :cw], ps[:, :cw], 0.0, None,
                                            ALU.max, ALU.add,
                                            accum_out=pU[:, ci:ci + 1])
                biasU = act_bias(pU, s0u, f"U_{t}")

                # ---- upd_align -> tanh -> state update, fused with the
                # next step's gate_w chunk-by-chunk (gate_w(t+1, ci) reads
                # Spack[ci] right after the update writes it) ---------------
                nxt = t + 1 < t_steps
                if nxt:
                    pHz, pHr = new_partials(t + 1)
                for ci, (cs, cw) in enumerate(chunks):
                    ps = psum.tile([128, chunk], f32, tag="ps",
                                   name=f"ps_u2_{t}_{ci}")
                    mm_bank(ps, U2, cs, cw, Cpack, t)
                    hc = hcpool.tile([128, chunk], bf16, tag="hc",
                                     name=f"hc_{t}_{ci}")
                    nc.scalar.activation(hc[:, :cw], ps[:, :cw], AF.Tanh,
                                         bias=biasU[:, 0:1])
                    # state = hc + r * (state - hc)
                    dd = dpool.tile([128, chunk], bf16, tag="dd",
                                    name=f"dd_{t}_{ci}")
                    nc.vector.tensor_sub(dd[:, :cw], Spack[:, cs:cs + cw],
                                         hc[:, :cw])
                    nc.vector.tensor_mul(dd[:, :cw], Rpack[:, cs:cs + cw],
                                         dd[:, :cw])
                    nc.vector.tensor_add(Spack[:, cs:cs + cw], hc[:, :cw],
                                         dd[:, :cw])
                    if nxt:
                        gate_w_chunk(t + 1, ci, cs, cw, pHz, pHr)

                nc.sync.dma_start(out_d[0, t], Spack[0:64, :])
                nc.sync.dma_start(out_d[1, t], Spack[64:128, :])
                # prefetch x two steps ahead (same parity slots, whose
                # readers in this step are done by now)
                if t + 2 < t_steps:
                    dma_x(t + 2)

    nc.compile()
    return nc


_PROG_CACHE = {}


def _get_program(n, t_steps, chunk, s0g, s0u):
    key = (n, t_steps, chunk, float(s0g), float(s0u))
    if key not in _PROG_CACHE:
        _PROG_CACHE[key] = _build_program(n, t_steps, chunk, s0g, s0u)
    return _PROG_CACHE[key]


# ---------------------------------------------------------------------------
# host-side packing / fallback
# ---------------------------------------------------------------------------

def _pack_weights(kw):
    """wst [64, 6, 64] (state rows) / wx1 [3, 6, 64] (x rows + bias row).
    Banks: Hz, Hr = gate_w halves; Z, R = gate_align halves; U1 = upd_w;
    U2 = upd_align."""
    wst = np.empty((64, 6, 64), np.float32)
    wx1 = np.empty((3, 6, 64), np.float32)
    specs = [
        (HZ, kw['gate_w'][:, 0:64], kw['gate_b'][0:64]),
        (HR, kw['gate_w'][:, 64:128], kw['gate_b'][64:128]),
        (ZB, kw['gate_align_w'][:, 0:64], kw['gate_align_b'][0:64]),
        (RB, kw['gate_align_w'][:, 64:128], kw['gate_align_b'][64:128]),
        (U1, kw['upd_w'], kw['upd_b']),
        (U2, kw['upd_align_w'], kw['upd_align_b']),
    ]
    for k, w, bias in specs:
        wst[:, k, :] = w[DIN:]
        wx1[0:DIN, k, :] = w[:DIN]
        wx1[DIN, k, :] = bias
    return wst, wx1


def _numpy_reference(x, init_state, kw):
    """Faithful general fallback (and testing oracle)."""
    def gfs(inp, aw, ab, w, b, nw, adw, afw, afb):
        res = inp @ aw + ab
        h = np.maximum(inp @ w + b, 0.0)
        s = adw[:, 0] * nw[0, :]
        agg = h.sum(axis=1)
        diff = s[None, :, None] * agg[:, None, :]
        return res + afw * diff + afb

    state = init_state[0].astype(np.float32)
    states = []
    for t in range(x.shape[1]):
        xt = x[:, t]
        inp = np.concatenate([xt, state], axis=-1)
        zr = 1.0 / (1.0 + np.exp(-gfs(inp, kw['gate_align_w'], kw['gate_align_b'],
                                      kw['gate_w'], kw['gate_b'], kw['gate_node_w'],
                                      kw['gate_add_w'], kw['gate_aff_w'],
                                      kw['gate_aff_b'])))
        z, r = zr[..., :H], zr[..., H:]
        cand = np.concatenate([xt, z * state], axis=-1)
        hc = np.tanh(gfs(cand, kw['upd_align_w'], kw['upd_align_b'],
                         kw['upd_w'], kw['upd_b'], kw['upd_node_w'],
                         kw['upd_add_w'], kw['upd_aff_w'], kw['upd_aff_b']))
        state = r * state + (1.0 - r) * hc
        states.append(state.copy())
    out = np.stack(states, axis=1).astype(np.float32)
    return out, np.ascontiguousarray(out[:, -1])


def kernel(**inputs):
    x = np.ascontiguousarray(np.asarray(inputs['x'], np.float32))
    init_state = np.asarray(inputs['init_state'], np.float32)
    kw = {k: np.asarray(v, np.float32) for k, v in inputs.items()
          if k not in ('x', 'init_state', 'node_emb0', 'node_emb1')}

    b_, t_, n_, d_ = x.shape

    s_g = kw['gate_add_w'][:, 0] * kw['gate_node_w'][0, :]
    s_u = kw['upd_add_w'][:, 0] * kw['upd_node_w'][0, :]

    def _uniform(v):
        return np.all(v == v.flat[0])

    structured = (
        b_ == B and t_ == T and n_ == N and d_ == DIN
        and _uniform(s_g) and _uniform(s_u)
        and np.all(kw['gate_aff_w'] == 1.0) and np.all(kw['gate_aff_b'] == 0.0)
        and np.all(kw['upd_aff_w'] == 1.0) and np.all(kw['upd_aff_b'] == 0.0)
    )
    if not structured:
        return _numpy_reference(x, init_state, kw)

    s0g = float(s_g.flat[0])
    s0u = float(s_u.flat[0])

    import ml_dtypes
    bf = ml_dtypes.bfloat16
    wst, wx1 = _pack_weights(kw)
    wst = wst.astype(bf)
    wx1 = wx1.astype(bf)

    xt_all = np.ascontiguousarray(x.transpose(0, 1, 3, 2)).astype(bf)
    st0_all = np.ascontiguousarray(init_state[0].transpose(0, 2, 1)).astype(bf)

    chunk = 1024
    nc = _get_program(N, T, chunk, s0g, s0u)

    from concourse.bass_utils import run_bass_kernel_spmd
    in_maps = []
    for c in range(NCORES):
        sl = slice(BPC * c, BPC * (c + 1))
        in_maps.append(dict(xt=xt_all[sl], st0=st0_all[sl], wst=wst, wx1=wx1))

    trace = os.environ.get("DGCRM_TRACE", "0") == "1"
    res = run_bass_kernel_spmd(nc, in_maps, core_ids=list(range(NCORES)),
                               trace=trace)
    if trace and res.exec_time_ns is not None:
        kernel._last_exec_time_ns = res.exec_time_ns
        kernel._last_trace = res.instructions_and_trace
    out_cm = np.concatenate([r["out_cm"] for r in res.results], axis=0)  # [B,T,64,N]
    states = np.ascontiguousarray(out_cm.transpose(0, 1, 3, 2).astype(np.float32))
    last = np.ascontiguousarray(states[:, -1])
    return states, last


kernel._last_exec_time_ns = None
kernel._last_trace = None
